# revision 26
# baseline (speedup 1.0000x reference)
"""Single-head causal self-attention on 8 trn2 NeuronCores.

B=16, T=4096, D=64 fp32. Data-parallel over batch: 2 batches per core.
Per core/batch: x -> xT (host transpose), QT/KT (dup'd weight proj), V
natural layout [keys, 64]; scores computed transposed (keys on partitions)
with 2x row-tiled pairs; exp on ScalarE (fused scale+bias) with every 4th
tile offloaded to the DVE via the fp16 Schraudolph bit trick; causal via
chunk skipping + N-restriction + affine_select on diagonal blocks; PV runs
in 4x column-tiled mode: each chunk pair becomes 4 concurrent M=32 matmuls
(even chunks -> PSUM rows 0:64, odd -> 64:128) plus M=1 column-tiled
denominator matmuls (ones weights -> PSUM rows 0/32/64/96); phases are
batch-interleaved so the PE always has dense work. Host sums the halves
and normalizes.
"""
import os
import sys

os.environ.setdefault("MYCRO_LOCAL_CACHE", "1")
sys.path.insert(0, "/opt/trn_rl_repo")

import numpy as np

import concourse.bass as bass
import concourse.tile as tile
from concourse import bacc, mybir
from concourse.bass_utils import run_bass_kernel_spmd

F32 = mybir.dt.float32
F16 = mybir.dt.float16

N_CORES = 8
B_LOC = 2          # batches per core
T = 4096
D = 64
NQ = 8             # q-superblocks of 512 per batch
QB = 512           # q-superblock width
KB = 128           # key chunk (PSUM partition dim of scores)
NCHUNK = T // KB   # 32 key chunks per batch
TPC = 2            # chunks per ST/PT tile (ACT granularity = 1024 cols)

# fp16 Schraudolph exp: bits = round(s*A + B) viewed as fp16
# equals exp(0.125*s - 8) * (1 +- ~3% sawtooth); A = 0.125*1024/ln2,
# B = 15360 - 8*1024/ln2 - 59.3 (balanced piecewise-linear offset)
SCHRAUD_A = 184.66501
SCHRAUD_B = 3481.73


def _build():
    nc = bacc.Bacc(None)

    xt_d = nc.declare_dram_parameter("xt16", [B_LOC, 128, T], F16, isOutput=False)
    wq2_d = nc.declare_dram_parameter("wq2", [128, 128], F32, isOutput=False)
    wk2_d = nc.declare_dram_parameter("wk2", [128, 128], F32, isOutput=False)
    wv_d = nc.declare_dram_parameter("wv", [128, D], F32, isOutput=False)
    id_d = nc.declare_dram_parameter("ident", [128, 128], F32, isOutput=False)
    yt_d = nc.declare_dram_parameter("ytn", [B_LOC, NQ, 97, QB], F32, isOutput=True)

    with tile.TileContext(nc) as tc:
        with (
            tc.tile_pool(name="consts", bufs=1) as consts,
            tc.tile_pool(name="xt", bufs=2) as xt_p,
            tc.tile_pool(name="qt", bufs=2) as qt_p,
            tc.tile_pool(name="kt", bufs=2) as kt_p,
            tc.tile_pool(name="v1", bufs=2) as v1_p,
            tc.tile_pool(name="pt", bufs=14) as pt_p,
            tc.tile_pool(name="scratch", bufs=4) as scratch_p,
            tc.tile_pool(name="stps", bufs=3, space="PSUM") as st_ps,
            tc.tile_pool(name="ytps", bufs=2, space="PSUM") as yt_ps,
        ):
            # ---- constants ----
            ident = consts.tile([128, 128], F16, tag="ident")
            nc.gpsimd.dma_start(out=ident, in_=id_d[:, :])
            wq2 = consts.tile([128, 128], F16, tag="wq2")
            nc.gpsimd.dma_start(out=wq2, in_=wq2_d[:, :])
            wk2 = consts.tile([128, 128], F16, tag="wk2")
            nc.gpsimd.dma_start(out=wk2, in_=wk2_d[:, :])
            wv = consts.tile([128, D], F16, tag="wv")
            nc.gpsimd.dma_start(out=wv, in_=wv_d[:, :])

            nbias = consts.tile([128, 1], F32, tag="nbias")
            nc.vector.memset(nbias, -8.0)
            ones = consts.tile([128, 1], F16, tag="ones")
            nc.vector.memset(ones, 1.0)

            # ---- warmups: ACT table load + PE HAM ramp ----
            wsc = scratch_p.tile([128, 128], F32, tag="wexp")
            nc.scalar.activation(out=wsc, in_=wq2, func=mybir.ActivationFunctionType.Exp, scale=0.01)
            for _ in range(8):
                wps = yt_ps.tile([128, QB], F32, tag="yt", name="wps")
                nc.tensor.matmul(out=wps[:, 0:128], lhsT=ident, rhs=ident, start=True, stop=True)

            state = {}

            def make_prologue(b):
                xt = xt_p.tile([128, T], F16, tag="xt", name="xt")
                for dj in range(NQ):
                    nc.sync.dma_start(
                        out=xt[:, QB * dj : QB * (dj + 1)],
                        in_=xt_d[b, :, QB * dj : QB * (dj + 1)],
                    )
                # Q/K projections (weights duplicated -> output rows 0:64 and 64:128)
                qt = qt_p.tile([128, T], F16, tag="qt", name="qt")
                kt = kt_p.tile([128, T], F16, tag="kt", name="kt")
                v1 = v1_p.tile([128, NCHUNK, D], F16, tag="v1", name="v1")
                state[b] = (qt, kt, v1)

                def proj_qk(j):
                    pq = yt_ps.tile([128, QB], F32, tag="yt", name="pq")
                    hq = 64 * (j % 2)
                    nc.tensor.matmul(out=pq, lhsT=wq2[hq : hq + 64, :], rhs=xt[hq : hq + 64, QB * j : QB * (j + 1)], start=True, stop=True)
                    nc.vector.tensor_copy(out=qt[:, QB * j : QB * (j + 1)], in_=pq)
                    pk = yt_ps.tile([128, QB], F32, tag="yt", name="pk")
                    nc.tensor.matmul(out=pk, lhsT=wk2[64 - hq : 128 - hq, :], rhs=xt[64 - hq : 128 - hq, QB * j : QB * (j + 1)], start=True, stop=True)
                    nc.vector.tensor_copy(out=kt[:, QB * j : QB * (j + 1)], in_=pk)

                def proj_v(g):
                    pvpa = yt_ps.tile([128, QB // 2], F32, tag="yt", name="pvpa")
                    pvpb = yt_ps.tile([128, QB // 2], F32, tag="yt", name="pvpb")
                    for k in range(8):
                        t = 8 * g + k
                        h = 64 * (k % 2)
                        dst = pvpa if k % 2 == 0 else pvpb
                        nc.tensor.matmul(
                            out=dst[:, D * (k // 2) : D * (k // 2 + 1)],
                            lhsT=xt[h : h + 64, 128 * t : 128 * (t + 1)],
                            rhs=wv[h : h + 64, :],
                            start=True,
                            stop=True,
                        )
                    nc.vector.tensor_copy(
                        out=v1[:, 8 * g : 8 * (g + 1) : 2, :],
                        in_=pvpa.rearrange("p (k c) -> p k c", c=D),
                    )
                    nc.vector.tensor_copy(
                        out=v1[:, 8 * g + 1 : 8 * (g + 1) : 2, :],
                        in_=pvpb.rearrange("p (k c) -> p k c", c=D),
                    )

                return proj_qk, proj_v

            def main_superblock(b, m):
                """v3-style per-tile pipeline: score pairs one tile ahead of
                exp, PV trailing; PV+den run as 3 concurrent column tiles
                (v dims 0:32 -> position 0, 32:64 -> 32, ones-denominator ->
                64) accumulating into one PSUM bank, rows 0:65."""
                qt, kt, v1 = state[b]
                nch = 4 * m + 4
                ntiles = nch // TPC
                yt = yt_ps.tile([128, QB], F32, tag="yt", name="ytacc")
                if m == 0:
                    # den96 region starts at chunk 1 (qoff=128 in m=0): zero
                    # the never-written columns
                    nc.vector.memset(yt[96:97, 0:128], 0.0)
                st_tiles = []
                pt_tiles = []

                def emit_st(ti):
                    st_t = st_ps.tile([128, QB * TPC], F32, tag="st", name="st_t")
                    st_tiles.append(st_t)
                    for slot in range(TPC):
                        c = TPC * ti + slot
                        j = c - 4 * m
                        qoff = 128 * j if j >= 0 else 0
                        half = 64 * (c % 2)
                        nc.tensor.matmul(
                            out=st_t[:, QB * slot + qoff : QB * (slot + 1)],
                            lhsT=kt[half : half + 64, KB * c : KB * (c + 1)],
                            rhs=qt[half : half + 64, QB * m + qoff : QB * (m + 1)],
                            start=True,
                            stop=True,
                        )

                def emit_act(ti):
                    st_t = st_tiles[ti]
                    pt = pt_p.tile([128, TPC, QB], F16, tag="pt", name="pt")
                    pt_tiles.append(pt)
                    use_dve = ti % 5 in (2, 4) and not os.environ.get("BASS_NO_DVE_EXP")

                    def emit_exp(dst, src_ap):
                        if use_dve:
                            nc.vector.tensor_scalar(
                                out=dst.bitcast(mybir.dt.uint16),
                                in0=src_ap,
                                scalar1=SCHRAUD_A,
                                scalar2=SCHRAUD_B,
                                op0=mybir.AluOpType.mult,
                                op1=mybir.AluOpType.add,
                            )
                        else:
                            nc.scalar.activation(
                                out=dst,
                                in_=src_ap,
                                func=mybir.ActivationFunctionType.Exp,
                                bias=nbias,
                                scale=0.125,
                            )

                    garbage = sum(
                        128 * (c - 4 * m)
                        for c in range(TPC * ti, TPC * ti + TPC)
                        if c - 4 * m > 0
                    )
                    if garbage >= (1 if os.environ.get("BASS_SIM_SAFE") else 384):
                        for slot in range(TPC):
                            c = TPC * ti + slot
                            j = c - 4 * m
                            qoff = 128 * j if j >= 0 else 0
                            emit_exp(
                                pt[:, slot, qoff:QB],
                                st_t[:, QB * slot + qoff : QB * (slot + 1)],
                            )
                    else:
                        emit_exp(pt.rearrange("p a b -> p (a b)"), st_t[:, : QB * TPC])
                    for slot in range(TPC):
                        c = TPC * ti + slot
                        j = c - 4 * m
                        if j >= 0:
                            sub = pt[:, slot, 128 * j : 128 * (j + 1)]
                            nc.gpsimd.affine_select(
                                out=sub,
                                in_=sub,
                                compare_op=mybir.AluOpType.is_ge,
                                fill=0.0,
                                base=0,
                                pattern=[[1, 128]],
                                channel_multiplier=-1,
                            )

                def emit_pv(ti):
                    pt = pt_tiles[ti]
                    for slot in range(TPC):
                        c = TPC * ti + slot
                        j = c - 4 * m
                        qoff = 128 * j if j >= 0 else 0
                        start = c == 0
                        stop = c == nch - 1
                        for h in range(2):
                            nc.tensor.matmul(
                                out=yt[32 * h : 32 * (h + 1), qoff:QB],
                                lhsT=v1[:, c, 32 * h : 32 * (h + 1)],
                                rhs=pt[:, slot, qoff:QB],
                                start=start,
                                stop=stop,
                                skip_group_check=True,
                                tile_position=(0, 32 * h),
                            )
                        # denominator alternates between column positions
                        # 64 and 96 so four array quadrants run concurrently
                        drow = 64 + 32 * (c % 2)
                        nc.tensor.matmul(
                            out=yt[drow : drow + 1, qoff:QB],
                            lhsT=ones,
                            rhs=pt[:, slot, qoff:QB],
                            start=c < 2,
                            stop=c >= nch - 2,
                            skip_group_check=True,
                            tile_position=(0, drow),
                        )

                # groups of up to 3 tiles per tiling-mode phase: score
                # pairs burst in row mode, PV+den burst in column mode, so
                # the PE pays mode-switch drains per group, not per tile
                G = 3
                groups = [list(range(g, min(g + G, ntiles))) for g in range(0, ntiles, G)]
                for ti in groups[0]:
                    emit_st(ti)
                for gi in range(1, len(groups)):
                    for ti in groups[gi - 1]:
                        emit_act(ti)
                    for ti in groups[gi]:
                        emit_st(ti)
                    for ti in groups[gi - 1]:
                        emit_pv(ti)
                for ti in groups[-1]:
                    emit_act(ti)
                for ti in groups[-1]:
                    emit_pv(ti)
                ytsb = scratch_p.tile([97, QB], F32, tag="ytsb", name="ytsb")
                nc.vector.tensor_copy(out=ytsb, in_=yt[0:97, :])
                nc.sync.dma_start(out=yt_d[b, m, :, :], in_=ytsb)

            # emission: batches interleaved superblock-by-superblock so the
            # PE always has dense matmul work; projections just-in-time.
            pq0, pv0 = make_prologue(0)
            pq1, pv1 = make_prologue(1)
            pq0(0); pv0(0); pq1(0); pv1(0)
            main_superblock(0, 0); pq0(1)
            main_superblock(1, 0); pq1(1)
            main_superblock(0, 1); pq0(2); pv0(1)
            main_superblock(1, 1); pq1(2); pv1(1)
            main_superblock(0, 2); pq0(3)
            main_superblock(1, 2); pq1(3)
            main_superblock(0, 3); pq0(4); pv0(2)
            main_superblock(1, 3); pq1(4); pv1(2)
            main_superblock(0, 4); pq0(5)
            main_superblock(1, 4); pq1(5)
            main_superblock(0, 5); pq0(6); pv0(3)
            main_superblock(1, 5); pq1(6); pv1(3)
            main_superblock(0, 6); pq0(7)
            main_superblock(1, 6); pq1(7)
            main_superblock(0, 7)
            main_superblock(1, 7)

    nc.finalize()
    return nc


_NC = None


def _get_nc():
    global _NC
    if _NC is None:
        _NC = _build()
    return _NC


def _run(x, Wk, Wq, Wv, trace=False):
    x = np.ascontiguousarray(np.asarray(x, dtype=np.float32))
    Wk = np.asarray(Wk, dtype=np.float32)
    Wq = np.asarray(Wq, dtype=np.float32)
    Wv = np.asarray(Wv, dtype=np.float32)
    B = x.shape[0]
    assert B == N_CORES * B_LOC and x.shape[1] == T and x.shape[2] == D

    wq2 = np.concatenate([Wq.T, Wq.T], axis=1)
    wq2 = np.ascontiguousarray(np.concatenate([wq2, wq2], axis=0))
    wk2 = np.concatenate([Wk.T, Wk.T], axis=1)
    wk2 = np.ascontiguousarray(np.concatenate([wk2, wk2], axis=0))
    wv = np.ascontiguousarray(np.concatenate([Wv.T, Wv.T], axis=0))
    ident = np.eye(128, dtype=np.float32)

    xt16 = x.astype(np.float16).transpose(0, 2, 1)
    xt16 = np.ascontiguousarray(np.concatenate([xt16, xt16], axis=1))
    in_maps = []
    for c in range(N_CORES):
        in_maps.append(
            {
                "xt16": np.ascontiguousarray(xt16[B_LOC * c : B_LOC * (c + 1)]),
                "wq2": wq2,
                "wk2": wk2,
                "wv": wv,
                "ident": ident,
            }
        )

    nc = _get_nc()
    res = run_bass_kernel_spmd(nc, in_maps, core_ids=list(range(N_CORES)), trace=trace)

    y = np.empty((B, T, D), dtype=np.float32)
    for c in range(N_CORES):
        ytn = res.results[c]["ytn"]  # [B_LOC, NQ, 97, 512]
        num = ytn[:, :, :D, :]
        den = ytn[:, :, 64:65, :] + ytn[:, :, 96:97, :]
        yb = (num / den).transpose(0, 1, 3, 2).reshape(B_LOC, T, D)
        y[B_LOC * c : B_LOC * (c + 1)] = yb
    return y, res


def kernel(x, Wk, Wq, Wv):
    y, _ = _run(x, Wk, Wq, Wv, trace=False)
    return y


# revision 27
# speedup vs baseline: 1.0642x; 1.0642x over previous
"""Single-head causal self-attention on 8 trn2 NeuronCores.

B=16, T=4096, D=64 fp32. Data-parallel over batch: 2 batches per core.
Per core/batch: x -> xT (host transpose), QT/KT (dup'd weight proj), V
natural layout [keys, 64]; scores computed transposed (keys on partitions)
with 2x row-tiled pairs; exp on ScalarE (fused scale+bias) with every 4th
tile offloaded to the DVE via the fp16 Schraudolph bit trick; causal via
chunk skipping + N-restriction + affine_select on diagonal blocks; PV runs
in 4x column-tiled mode: each chunk pair becomes 4 concurrent M=32 matmuls
(even chunks -> PSUM rows 0:64, odd -> 64:128) plus M=1 column-tiled
denominator matmuls (ones weights -> PSUM rows 0/32/64/96); phases are
batch-interleaved so the PE always has dense work. Host sums the halves
and normalizes.
"""
import os
import sys

os.environ.setdefault("MYCRO_LOCAL_CACHE", "1")
sys.path.insert(0, "/opt/trn_rl_repo")

import numpy as np

import concourse.bass as bass
import concourse.tile as tile
from concourse import bacc, mybir
from concourse.bass_utils import run_bass_kernel_spmd

F32 = mybir.dt.float32
F16 = mybir.dt.float16

N_CORES = 8
B_LOC = 2          # batches per core
T = 4096
D = 64
NQ = 8             # q-superblocks of 512 per batch
QB = 512           # q-superblock width
KB = 128           # key chunk (PSUM partition dim of scores)
NCHUNK = T // KB   # 32 key chunks per batch
TPC = 2            # chunks per ST/PT tile (ACT granularity = 1024 cols)

# fp16 Schraudolph exp: bits = round(s*A + B) viewed as fp16
# equals exp(0.125*s - 8) * (1 +- ~3% sawtooth); A = 0.125*1024/ln2,
# B = 15360 - 8*1024/ln2 - 59.3 (balanced piecewise-linear offset)
SCHRAUD_A = 184.66501
SCHRAUD_B = 3481.73


def _build():
    nc = bacc.Bacc(None)

    xt_d = nc.declare_dram_parameter("xt16", [B_LOC, 128, T], F16, isOutput=False)
    wq2_d = nc.declare_dram_parameter("wq2", [128, 128], F32, isOutput=False)
    wk2_d = nc.declare_dram_parameter("wk2", [128, 128], F32, isOutput=False)
    wv_d = nc.declare_dram_parameter("wv", [128, D], F32, isOutput=False)
    id_d = nc.declare_dram_parameter("ident", [128, 128], F32, isOutput=False)
    yt_d = nc.declare_dram_parameter("ytn", [B_LOC, NQ, D + 1, QB], F32, isOutput=True)

    with tile.TileContext(nc) as tc:
        with (
            tc.tile_pool(name="consts", bufs=1) as consts,
            tc.tile_pool(name="xt", bufs=2) as xt_p,
            tc.tile_pool(name="qt", bufs=2) as qt_p,
            tc.tile_pool(name="kt", bufs=2) as kt_p,
            tc.tile_pool(name="v1", bufs=2) as v1_p,
            tc.tile_pool(name="pt", bufs=14) as pt_p,
            tc.tile_pool(name="scratch", bufs=4) as scratch_p,
            tc.tile_pool(name="stps", bufs=3, space="PSUM") as st_ps,
            tc.tile_pool(name="ytps", bufs=2, space="PSUM") as yt_ps,
        ):
            # ---- constants ----
            ident = consts.tile([128, 128], F16, tag="ident")
            nc.gpsimd.dma_start(out=ident, in_=id_d[:, :])
            wq2 = consts.tile([128, 128], F16, tag="wq2")
            nc.gpsimd.dma_start(out=wq2, in_=wq2_d[:, :])
            wk2 = consts.tile([128, 128], F16, tag="wk2")
            nc.gpsimd.dma_start(out=wk2, in_=wk2_d[:, :])
            wv = consts.tile([128, D], F16, tag="wv")
            nc.gpsimd.dma_start(out=wv, in_=wv_d[:, :])

            nbias = consts.tile([128, 1], F32, tag="nbias")
            nc.vector.memset(nbias, -8.0)
            ones = consts.tile([128, 1], F16, tag="ones")
            nc.vector.memset(ones, 1.0)

            # ---- warmups: ACT table load + PE HAM ramp ----
            wsc = scratch_p.tile([128, 128], F32, tag="wexp")
            nc.scalar.activation(out=wsc, in_=wq2, func=mybir.ActivationFunctionType.Exp, scale=0.01)
            for _ in range(8):
                wps = yt_ps.tile([128, QB], F32, tag="yt", name="wps")
                nc.tensor.matmul(out=wps[:, 0:128], lhsT=ident, rhs=ident, start=True, stop=True)

            state = {}

            def make_prologue(b):
                xt = xt_p.tile([128, T], F16, tag="xt", name="xt")
                for dj in range(NQ):
                    nc.sync.dma_start(
                        out=xt[:, QB * dj : QB * (dj + 1)],
                        in_=xt_d[b, :, QB * dj : QB * (dj + 1)],
                    )
                # Q/K projections (weights duplicated -> output rows 0:64 and 64:128)
                qt = qt_p.tile([128, T], F16, tag="qt", name="qt")
                kt = kt_p.tile([128, T], F16, tag="kt", name="kt")
                v1 = v1_p.tile([128, NCHUNK, D], F16, tag="v1", name="v1")
                state[b] = (qt, kt, v1)

                def proj_qk(j):
                    pq = yt_ps.tile([128, QB], F32, tag="yt", name="pq")
                    hq = 64 * (j % 2)
                    nc.tensor.matmul(out=pq, lhsT=wq2[hq : hq + 64, :], rhs=xt[hq : hq + 64, QB * j : QB * (j + 1)], start=True, stop=True)
                    nc.vector.tensor_copy(out=qt[:, QB * j : QB * (j + 1)], in_=pq)
                    pk = yt_ps.tile([128, QB], F32, tag="yt", name="pk")
                    nc.tensor.matmul(out=pk, lhsT=wk2[64 - hq : 128 - hq, :], rhs=xt[64 - hq : 128 - hq, QB * j : QB * (j + 1)], start=True, stop=True)
                    nc.vector.tensor_copy(out=kt[:, QB * j : QB * (j + 1)], in_=pk)

                def proj_v(g):
                    pvpa = yt_ps.tile([128, QB // 2], F32, tag="yt", name="pvpa")
                    pvpb = yt_ps.tile([128, QB // 2], F32, tag="yt", name="pvpb")
                    for k in range(8):
                        t = 8 * g + k
                        h = 64 * (k % 2)
                        dst = pvpa if k % 2 == 0 else pvpb
                        nc.tensor.matmul(
                            out=dst[:, D * (k // 2) : D * (k // 2 + 1)],
                            lhsT=xt[h : h + 64, 128 * t : 128 * (t + 1)],
                            rhs=wv[h : h + 64, :],
                            start=True,
                            stop=True,
                        )
                    nc.vector.tensor_copy(
                        out=v1[:, 8 * g : 8 * (g + 1) : 2, :],
                        in_=pvpa.rearrange("p (k c) -> p k c", c=D),
                    )
                    nc.vector.tensor_copy(
                        out=v1[:, 8 * g + 1 : 8 * (g + 1) : 2, :],
                        in_=pvpb.rearrange("p (k c) -> p k c", c=D),
                    )

                return proj_qk, proj_v

            def main_superblock(b, m):
                """v3-style per-tile pipeline: score pairs one tile ahead of
                exp, PV trailing; PV+den run as 3 concurrent column tiles
                (v dims 0:32 -> position 0, 32:64 -> 32, ones-denominator ->
                64) accumulating into one PSUM bank, rows 0:65."""
                qt, kt, v1 = state[b]
                nch = 4 * m + 4
                ntiles = nch // TPC
                yt = yt_ps.tile([128, QB], F32, tag="yt", name="ytacc")
                st_tiles = []
                pt_tiles = []

                def emit_st(ti):
                    st_t = st_ps.tile([128, QB * TPC], F32, tag="st", name="st_t")
                    st_tiles.append(st_t)
                    for slot in range(TPC):
                        c = TPC * ti + slot
                        j = c - 4 * m
                        qoff = 128 * j if j >= 0 else 0
                        half = 64 * (c % 2)
                        nc.tensor.matmul(
                            out=st_t[:, QB * slot + qoff : QB * (slot + 1)],
                            lhsT=kt[half : half + 64, KB * c : KB * (c + 1)],
                            rhs=qt[half : half + 64, QB * m + qoff : QB * (m + 1)],
                            start=True,
                            stop=True,
                        )

                def emit_act(ti):
                    st_t = st_tiles[ti]
                    pt = pt_p.tile([128, TPC, QB], F16, tag="pt", name="pt")
                    pt_tiles.append(pt)
                    use_dve = ti % 5 in (2, 4) and not os.environ.get("BASS_NO_DVE_EXP")

                    def emit_exp(dst, src_ap):
                        if use_dve:
                            nc.vector.tensor_scalar(
                                out=dst.bitcast(mybir.dt.uint16),
                                in0=src_ap,
                                scalar1=SCHRAUD_A,
                                scalar2=SCHRAUD_B,
                                op0=mybir.AluOpType.mult,
                                op1=mybir.AluOpType.add,
                            )
                        else:
                            nc.scalar.activation(
                                out=dst,
                                in_=src_ap,
                                func=mybir.ActivationFunctionType.Exp,
                                bias=nbias,
                                scale=0.125,
                            )

                    garbage = sum(
                        128 * (c - 4 * m)
                        for c in range(TPC * ti, TPC * ti + TPC)
                        if c - 4 * m > 0
                    )
                    if garbage >= (1 if os.environ.get("BASS_SIM_SAFE") else 384):
                        for slot in range(TPC):
                            c = TPC * ti + slot
                            j = c - 4 * m
                            qoff = 128 * j if j >= 0 else 0
                            emit_exp(
                                pt[:, slot, qoff:QB],
                                st_t[:, QB * slot + qoff : QB * (slot + 1)],
                            )
                    else:
                        emit_exp(pt.rearrange("p a b -> p (a b)"), st_t[:, : QB * TPC])
                    for slot in range(TPC):
                        c = TPC * ti + slot
                        j = c - 4 * m
                        if j >= 0:
                            sub = pt[:, slot, 128 * j : 128 * (j + 1)]
                            nc.gpsimd.affine_select(
                                out=sub,
                                in_=sub,
                                compare_op=mybir.AluOpType.is_ge,
                                fill=0.0,
                                base=0,
                                pattern=[[1, 128]],
                                channel_multiplier=-1,
                            )

                def emit_pv(ti):
                    pt = pt_tiles[ti]
                    for slot in range(TPC):
                        c = TPC * ti + slot
                        j = c - 4 * m
                        qoff = 128 * j if j >= 0 else 0
                        start = c == 0
                        stop = c == nch - 1
                        for h in range(2):
                            nc.tensor.matmul(
                                out=yt[32 * h : 32 * (h + 1), qoff:QB],
                                lhsT=v1[:, c, 32 * h : 32 * (h + 1)],
                                rhs=pt[:, slot, qoff:QB],
                                start=start,
                                stop=stop,
                                skip_group_check=True,
                                tile_position=(0, 32 * h),
                            )
                        nc.tensor.matmul(
                            out=yt[64:65, qoff:QB],
                            lhsT=ones,
                            rhs=pt[:, slot, qoff:QB],
                            start=start,
                            stop=stop,
                            skip_group_check=True,
                            tile_position=(0, 64),
                        )

                # groups of up to 3 tiles per tiling-mode phase: score
                # pairs burst in row mode, PV+den burst in column mode, so
                # the PE pays mode-switch drains per group, not per tile
                G = 3
                groups = [list(range(g, min(g + G, ntiles))) for g in range(0, ntiles, G)]
                for ti in groups[0]:
                    emit_st(ti)
                for gi in range(1, len(groups)):
                    for ti in groups[gi - 1]:
                        emit_act(ti)
                    for ti in groups[gi]:
                        emit_st(ti)
                    for ti in groups[gi - 1]:
                        emit_pv(ti)
                for ti in groups[-1]:
                    emit_act(ti)
                for ti in groups[-1]:
                    emit_pv(ti)
                ytsb = scratch_p.tile([D + 1, QB], F32, tag="ytsb", name="ytsb")
                nc.vector.tensor_copy(out=ytsb, in_=yt[0 : D + 1, :])
                nc.sync.dma_start(out=yt_d[b, m, :, :], in_=ytsb)

            # emission: batches interleaved superblock-by-superblock so the
            # PE always has dense matmul work; projections just-in-time.
            pq0, pv0 = make_prologue(0)
            pq1, pv1 = make_prologue(1)
            pq0(0); pv0(0); pq1(0); pv1(0)
            main_superblock(0, 0); pq0(1)
            main_superblock(1, 0); pq1(1)
            main_superblock(0, 1); pq0(2); pv0(1)
            main_superblock(1, 1); pq1(2); pv1(1)
            main_superblock(0, 2); pq0(3)
            main_superblock(1, 2); pq1(3)
            main_superblock(0, 3); pq0(4); pv0(2)
            main_superblock(1, 3); pq1(4); pv1(2)
            main_superblock(0, 4); pq0(5)
            main_superblock(1, 4); pq1(5)
            main_superblock(0, 5); pq0(6); pv0(3)
            main_superblock(1, 5); pq1(6); pv1(3)
            main_superblock(0, 6); pq0(7)
            main_superblock(1, 6); pq1(7)
            main_superblock(0, 7)
            main_superblock(1, 7)

    nc.finalize()
    return nc


_NC = None


def _get_nc():
    global _NC
    if _NC is None:
        _NC = _build()
    return _NC


def _run(x, Wk, Wq, Wv, trace=False):
    x = np.ascontiguousarray(np.asarray(x, dtype=np.float32))
    Wk = np.asarray(Wk, dtype=np.float32)
    Wq = np.asarray(Wq, dtype=np.float32)
    Wv = np.asarray(Wv, dtype=np.float32)
    B = x.shape[0]
    assert B == N_CORES * B_LOC and x.shape[1] == T and x.shape[2] == D

    wq2 = np.concatenate([Wq.T, Wq.T], axis=1)
    wq2 = np.ascontiguousarray(np.concatenate([wq2, wq2], axis=0))
    wk2 = np.concatenate([Wk.T, Wk.T], axis=1)
    wk2 = np.ascontiguousarray(np.concatenate([wk2, wk2], axis=0))
    wv = np.ascontiguousarray(np.concatenate([Wv.T, Wv.T], axis=0))
    ident = np.eye(128, dtype=np.float32)

    xt16 = x.astype(np.float16).transpose(0, 2, 1)
    xt16 = np.ascontiguousarray(np.concatenate([xt16, xt16], axis=1))
    in_maps = []
    for c in range(N_CORES):
        in_maps.append(
            {
                "xt16": np.ascontiguousarray(xt16[B_LOC * c : B_LOC * (c + 1)]),
                "wq2": wq2,
                "wk2": wk2,
                "wv": wv,
                "ident": ident,
            }
        )

    nc = _get_nc()
    res = run_bass_kernel_spmd(nc, in_maps, core_ids=list(range(N_CORES)), trace=trace)

    y = np.empty((B, T, D), dtype=np.float32)
    for c in range(N_CORES):
        ytn = res.results[c]["ytn"]  # [B_LOC, NQ, 65, 512]
        num = ytn[:, :, :D, :]
        den = ytn[:, :, D : D + 1, :]
        yb = (num / den).transpose(0, 1, 3, 2).reshape(B_LOC, T, D)
        y[B_LOC * c : B_LOC * (c + 1)] = yb
    return y, res


def kernel(x, Wk, Wq, Wv):
    y, _ = _run(x, Wk, Wq, Wv, trace=False)
    return y
